# revision 28
# baseline (speedup 1.0000x reference)
"""Trainium2 Bass/Tile kernel for per-patch multi-head attention.

Problem: x [B=4, P=4, N=1024, C=512]; per-patch Wq [P, C, C], Wkv [P, C, 2C];
shared Wproj [C, C], bproj [C]. 8 heads, hd=64.

Sharding: the 16 (b, p) pairs are fully independent; each of the 8 cores
processes 2 pairs (data/expert parallel, no collectives). Wproj/bproj are
replicated.

Per-core layout strategy (all matmul operands bf16, accum fp32):
  - x is cast to bf16 on DVE, then xT [c, n] via bf16 PE-transposes; the four
    128-wide transposes of an n-tile share one [128, 512] PSUM tile drained by
    a single strided DVE copy.
  - qT/kT [d, n] = W.T-contracted against xT (d head-major).
  - vpad [m, 8*65]: per-head 64 v columns + a ones column (gives softmax
    denominators for free in the av matmul).
  - scoresT [m, n] per head via K=64 matmuls; head pairs sit at partition
    offsets 0/64 so the PE row-tiles them concurrently.
  - exp on the scalar engine straight out of PSUM in [128, 2048] slabs.
  - av phase streams et (512-wide) against a stationary vpad head-slice
    [K=128, M=65]: out[d|den, n] accumulates over m-chunks. This produces
    oT head-major directly (no output transposes) with long PE streams.
  - normalize: fast-approx reciprocal of the denominator row (DVE), Pool
    partition_broadcast across 64 partitions, DVE scalar_tensor_tensor
    multiply casting to bf16 oT tiles.
  - proj consumes oT slices as stationary weights; bias via a K=1 ones-row
    matmul seeding the PSUM accumulation.
"""

import numpy as np

import concourse.bass as bass
import concourse.bacc as bacc
import concourse.mybir as mybir
from concourse.masks import make_identity
from concourse.tile import TileContext

B, P, N, C = 4, 4, 1024, 512
HEADS = 8
HD = C // HEADS  # 64
NT = N // 128  # 8 n-tiles
CCH = C // 128  # 4 c-chunks
F32 = mybir.dt.float32
BF16 = mybir.dt.bfloat16
MUL = mybir.AluOpType.mult

_CACHE = {}


def _build_kernel():
    nc = bacc.Bacc()
    x = nc.declare_dram_parameter("x", [2, N, C], F32, False)
    wq = nc.declare_dram_parameter("wq", [2, C, C], F32, False)
    wkv = nc.declare_dram_parameter("wkv", [2, C, 2 * C], F32, False)
    wproj = nc.declare_dram_parameter("wproj", [C, C], F32, False)
    bproj = nc.declare_dram_parameter("bproj", [1, C], F32, False)
    y = nc.declare_dram_parameter("y", [2, N, C], F32, True)

    with TileContext(nc) as tc:
        with (
            tc.tile_pool(name="consts", bufs=1) as consts,
            tc.tile_pool(name="wpool", bufs=2) as wpool,
            tc.tile_pool(name="xload", bufs=3) as xload,
            tc.tile_pool(name="bigp", bufs=2) as bigp,
            tc.tile_pool(name="expp", bufs=32) as expp,
            tc.tile_pool(name="smallp", bufs=4) as smallp,
            tc.tile_pool(name="otp", bufs=1) as otp,
            tc.tile_pool(name="ps_slab", bufs=2, space="PSUM") as ps_slab,
            tc.tile_pool(name="ps_av", bufs=2, space="PSUM") as ps_av,
            tc.tile_pool(name="ps_mm", bufs=1, space="PSUM") as ps_mm,
        ):
            identbf = consts.tile([128, 128], BF16)
            make_identity(nc, identbf)
            ones_bf = consts.tile([1, 128], BF16)
            nc.vector.memset(ones_bf, 1.0)

            wproj_sb = []
            for ci in range(CCH):
                t32 = xload.tile([128, 512], F32, tag="wload", name="wload")
                nc.gpsimd.dma_start(out=t32, in_=wproj[ci * 128 : (ci + 1) * 128, :])
                tb = consts.tile([128, 512], BF16, tag=f"wproj{ci}", name=f"wproj{ci}")
                nc.scalar.copy(tb, t32)
                wproj_sb.append(tb)
            bp32 = consts.tile([1, 512], F32)
            nc.gpsimd.dma_start(out=bp32, in_=bproj[:, :])
            bp_bf = consts.tile([1, 512], BF16)
            nc.vector.tensor_copy(bp_bf, bp32)

            for pr in range(2):
                # ---- kick off all x DMAs, then weight DMAs
                xt32s = []
                for nt in range(NT):
                    xt32 = xload.tile([128, 512], F32, tag="xload")
                    nc.gpsimd.dma_start(out=xt32, in_=x[pr, nt * 128 : (nt + 1) * 128, :])
                    xt32s.append(xt32)
                wq_sb, wk_sb, wv_sb = [], [], []
                w32s = []
                for ci in range(CCH):
                    rows = slice(ci * 128, (ci + 1) * 128)
                    for lst, wsrc in (
                        (wq_sb, wq[pr, rows, :]),
                        (wk_sb, wkv[pr, rows, 0:512]),
                        (wv_sb, wkv[pr, rows, 512:1024]),
                    ):
                        t32 = xload.tile([128, 512], F32, tag="wload", name="wload")
                        nc.gpsimd.dma_start(out=t32, in_=wsrc)
                        w32s.append(t32)

                # ---- xT [c-chunk, ci, n] via bf16 PE transpose
                xT = bigp.tile([128, CCH, N], BF16, tag="xTall", name="xTall")
                for nt in range(NT):
                    xtb = xload.tile([128, 512], BF16, tag="xbf")
                    nc.vector.tensor_copy(xtb, xt32s[nt])
                    for ci in range(CCH):
                        pst = ps_mm.tile([128, 128], BF16, tag=f"mm{pr}")
                        nc.tensor.transpose(
                            pst,
                            xtb[:, ci * 128 : (ci + 1) * 128],
                            identbf,
                        )
                        nc.vector.tensor_copy(
                            xT[:, ci, nt * 128 : (nt + 1) * 128], pst
                        )

                # ---- weight casts: wq/wk on DVE (gate qkT), wv on ACT
                for ci in range(CCH):
                    for j, (lst, tag, eng) in enumerate((
                        (wq_sb, f"wq{ci}", nc.vector),
                        (wk_sb, f"wk{ci}", nc.vector),
                        (wv_sb, f"wv{ci}", nc.scalar),
                    )):
                        t32 = w32s[3 * ci + j]
                        tb = wpool.tile([128, 512], BF16, tag=tag, name=tag)
                        if eng is nc.scalar:
                            nc.scalar.copy(tb, t32)
                        else:
                            eng.tensor_copy(tb, t32)
                        lst.append(tb)

                # ---- qT/kT [d, n] (d head-major: d-chunk di = heads 2di, 2di+1)
                qT = [bigp.tile([128, N], BF16, tag=f"qT{di}", name=f"qT{di}") for di in range(CCH)]
                kT = [bigp.tile([128, N], BF16, tag=f"kT{di}", name=f"kT{di}") for di in range(CCH)]
                for di in range(CCH):
                    dcols = slice(di * 128, (di + 1) * 128)
                    for nf in range(2):
                        ncols = slice(nf * 512, (nf + 1) * 512)
                        for dst, wsb in ((qT, wq_sb), (kT, wk_sb)):
                            ps = ps_mm.tile([128, 512], F32, tag=f"mm{pr}")
                            for ci in range(CCH):
                                nc.tensor.matmul(
                                    ps,
                                    wsb[ci][:, dcols],
                                    xT[:, ci, ncols],
                                    start=(ci == 0),
                                    stop=(ci == CCH - 1),
                                )
                            nc.vector.tensor_copy(dst[di][:, ncols], ps)

                # ---- v [m, d] padded with a ones column per head block
                vpad = [bigp.tile([128, HEADS * 65], BF16, tag=f"v{mt}", name=f"v{mt}") for mt in range(NT)]
                for mt in range(NT):
                    ps = ps_mm.tile([128, 512], F32, tag=f"mm{pr}")
                    for ci in range(CCH):
                        nc.tensor.matmul(
                            ps,
                            xT[:, ci, mt * 128 : (mt + 1) * 128],
                            wv_sb[ci],
                            start=(ci == 0),
                            stop=(ci == CCH - 1),
                        )
                    vv = vpad[mt].rearrange("p (h w) -> p h w", w=65)
                    nc.vector.memset(vv[:, :, 64:65], 1.0)
                    nc.vector.tensor_copy(
                        vv[:, :, 0:64], ps.rearrange("p (h w) -> p h w", w=64)
                    )

                # ---- attention: scores+exp per (di, mt); av streams et wide
                oT_sb = [otp.tile([128, N], BF16, tag=f"oT{di}", name=f"oT{di}") for di in range(CCH)]
                for di in range(CCH):
                    exps = [[None, None] for _ in range(NT)]
                    for mt in range(NT):
                        for half in range(2):
                            prow = slice(half * 64, (half + 1) * 64)
                            slab = ps_slab.tile([128, 1024], F32, tag="slab")
                            for nf in range(2):
                                nc.tensor.matmul(
                                    slab[:, nf * 512 : (nf + 1) * 512],
                                    kT[di][prow, mt * 128 : (mt + 1) * 128],
                                    qT[di][prow, nf * 512 : (nf + 1) * 512],
                                    start=True,
                                    stop=True,
                                )
                            et = expp.tile([128, 1024], BF16, tag="exp")
                            nc.scalar.activation(
                                et, slab, mybir.ActivationFunctionType.Exp, scale=0.125
                            )
                            exps[mt][half] = et

                    # av: out[d|den, n] = vpad_h.T @ et ; accumulate over mt.
                    # nf outer so the last di's nf-group unblocks proj of the
                    # matching n-tiles early. Denominator reciprocals are
                    # batched per (di, nf): the two halves' denom rows are
                    # PE-transposed into [128, 4] columns, reciprocal'd
                    # partition-parallel on DVE, and transposed back.
                    for nf in range(2):
                        avs = []
                        for half in range(2):
                            av = ps_av.tile([65, 512], F32, tag="av", name="av")
                            avs.append(av)
                        for mt in range(NT):
                            for half in range(2):
                                h = 2 * di + half
                                nc.tensor.matmul(
                                    avs[half],
                                    vpad[mt][:, h * 65 : (h + 1) * 65],
                                    exps[mt][half][
                                        :, nf * 512 : (nf + 1) * 512
                                    ],
                                    start=(mt == 0),
                                    stop=(mt == NT - 1),
                                )
                        dens = []
                        for half in range(2):
                            dn = smallp.tile([1, 512], BF16, tag=f"den{half}")
                            nc.vector.tensor_copy(dn, avs[half][64:65, :])
                            dens.append(dn)
                        rcT = ps_mm.tile([128, 16], BF16, tag=f"mm{pr}")
                        for half in range(2):
                            for k in range(4):
                                nc.tensor.transpose(
                                    rcT[:, 8 * half + 2 * k : 8 * half + 2 * k + 1],
                                    dens[half][:, 128 * k : 128 * (k + 1)],
                                    identbf[0:1, 0:1],
                                )
                        rcT_sb = smallp.tile([128, 8], BF16, tag="rcTsb")
                        with nc.allow_low_precision(
                            reason="bf16 softmax denom reciprocal"
                        ):
                            nc.vector.reciprocal(
                                rcT_sb,
                                rcT.rearrange("p (k two) -> p k two", two=2)[
                                    :, :, 0
                                ],
                            )
                        rc2s = []
                        for half in range(2):
                            r2 = ps_mm.tile([1, 512], BF16, tag=f"mm{pr}")
                            for k in range(4):
                                nc.tensor.transpose(
                                    r2[:, 128 * k : 128 * (k + 1)],
                                    rcT_sb[:, 4 * half + k : 4 * half + k + 1],
                                    identbf,
                                )
                            rc2s.append(r2)
                        for half in range(2):
                            rc = smallp.tile([1, 512], BF16, tag="rc")
                            nc.vector.tensor_copy(rc, rc2s[half])
                            repl = smallp.tile([64, 512], BF16, tag="repl")
                            nc.gpsimd.partition_broadcast(repl, rc)
                            nc.vector.scalar_tensor_tensor(
                                out=oT_sb[di][
                                    half * 64 : (half + 1) * 64,
                                    nf * 512 : (nf + 1) * 512,
                                ],
                                in0=avs[half][0:64, :],
                                scalar=1.0,
                                in1=repl,
                                op0=MUL,
                                op1=MUL,
                            )
                        # proj + bias for this nf's n-tiles once the last
                        # di's chains have landed
                        if di == CCH - 1:
                            for nt in range(nf * 4, nf * 4 + 4):
                                zps = ps_mm.tile([128, 512], F32, tag=f"mm{pr}")
                                nc.tensor.matmul(
                                    zps,
                                    ones_bf[0:1, :],
                                    bp_bf[0:1, :],
                                    start=True,
                                    stop=False,
                                )
                                for dj in range(CCH):
                                    nc.tensor.matmul(
                                        zps,
                                        oT_sb[dj][:, nt * 128 : (nt + 1) * 128],
                                        wproj_sb[dj],
                                        start=False,
                                        stop=(dj == CCH - 1),
                                    )
                                zsb = smallp.tile([128, 512], F32, tag="z")
                                if pr == 1:
                                    nc.scalar.copy(zsb, zps)
                                else:
                                    nc.vector.tensor_copy(zsb, zps)
                                nc.gpsimd.dma_start(
                                    out=y[pr, nt * 128 : (nt + 1) * 128, :],
                                    in_=zsb,
                                )
    return nc


def _get_nc():
    if "nc" not in _CACHE:
        nc = _build_kernel()
        nc.compile()
        _CACHE["nc"] = nc
    return _CACHE["nc"]


def kernel(**inputs) -> np.ndarray:
    from concourse.bass_utils import run_bass_kernel_spmd

    x = np.ascontiguousarray(np.asarray(inputs["x"], dtype=np.float32))
    Wq = np.ascontiguousarray(np.asarray(inputs["Wq"], dtype=np.float32))
    Wkv = np.ascontiguousarray(np.asarray(inputs["Wkv"], dtype=np.float32))
    Wproj = np.ascontiguousarray(np.asarray(inputs["Wproj"], dtype=np.float32))
    bproj = np.ascontiguousarray(
        np.asarray(inputs["bproj"], dtype=np.float32).reshape(1, C)
    )

    nc = _get_nc()
    xr = x.reshape(B * P, N, C)
    in_maps = []
    for core in range(8):
        p0 = (2 * core) % P
        in_maps.append(
            {
                "x": np.ascontiguousarray(xr[2 * core : 2 * core + 2]),
                "wq": np.ascontiguousarray(Wq[p0 : p0 + 2]),
                "wkv": np.ascontiguousarray(Wkv[p0 : p0 + 2]),
                "wproj": Wproj,
                "bproj": bproj,
            }
        )
    res = run_bass_kernel_spmd(nc, in_maps, list(range(8))).results
    out = np.concatenate([r["y"] for r in res], axis=0).reshape(B, P, N, C)
    return out.astype(np.float32)


# revision 31
# speedup vs baseline: 1.0014x; 1.0014x over previous
"""Trainium2 Bass/Tile kernel for per-patch multi-head attention.

Problem: x [B=4, P=4, N=1024, C=512]; per-patch Wq [P, C, C], Wkv [P, C, 2C];
shared Wproj [C, C], bproj [C]. 8 heads, hd=64.

Sharding: the 16 (b, p) pairs are fully independent; each of the 8 cores
processes 2 pairs (data/expert parallel, no collectives). Wproj/bproj are
replicated.

Per-core layout strategy (all matmul operands bf16, accum fp32):
  - x is cast to bf16 on DVE, then xT [c, n] via bf16 PE-transposes; the four
    128-wide transposes of an n-tile share one [128, 512] PSUM tile drained by
    a single strided DVE copy.
  - qT/kT [d, n] = W.T-contracted against xT (d head-major).
  - vpad [m, 8*65]: per-head 64 v columns + a ones column (gives softmax
    denominators for free in the av matmul).
  - scoresT [m, n] per head via K=64 matmuls; head pairs sit at partition
    offsets 0/64 so the PE row-tiles them concurrently.
  - exp on the scalar engine straight out of PSUM in [128, 2048] slabs.
  - av phase streams et (512-wide) against a stationary vpad head-slice
    [K=128, M=65]: out[d|den, n] accumulates over m-chunks. This produces
    oT head-major directly (no output transposes) with long PE streams.
  - normalize: fast-approx reciprocal of the denominator row (DVE), Pool
    partition_broadcast across 64 partitions, DVE scalar_tensor_tensor
    multiply casting to bf16 oT tiles.
  - proj consumes oT slices as stationary weights; bias via a K=1 ones-row
    matmul seeding the PSUM accumulation.
"""

import numpy as np

import concourse.bass as bass
import concourse.bacc as bacc
import concourse.mybir as mybir
from concourse.masks import make_identity
from concourse.tile import TileContext

B, P, N, C = 4, 4, 1024, 512
HEADS = 8
HD = C // HEADS  # 64
NT = N // 128  # 8 n-tiles
CCH = C // 128  # 4 c-chunks
F32 = mybir.dt.float32
BF16 = mybir.dt.bfloat16
MUL = mybir.AluOpType.mult

_CACHE = {}


def _build_kernel():
    nc = bacc.Bacc()
    x = nc.declare_dram_parameter("x", [2, N, C], F32, False)
    wq = nc.declare_dram_parameter("wq", [2, C, C], F32, False)
    wkv = nc.declare_dram_parameter("wkv", [2, C, 2 * C], F32, False)
    wproj = nc.declare_dram_parameter("wproj", [C, C], F32, False)
    bproj = nc.declare_dram_parameter("bproj", [1, C], F32, False)
    y = nc.declare_dram_parameter("y", [2, N, C], F32, True)

    with TileContext(nc) as tc:
        with (
            tc.tile_pool(name="consts", bufs=1) as consts,
            tc.tile_pool(name="wpool", bufs=2) as wpool,
            tc.tile_pool(name="xload", bufs=3) as xload,
            tc.tile_pool(name="bigp", bufs=2) as bigp,
            tc.tile_pool(name="expp", bufs=32) as expp,
            tc.tile_pool(name="smallp", bufs=4) as smallp,
            tc.tile_pool(name="otp", bufs=1) as otp,
            tc.tile_pool(name="ps_slab", bufs=2, space="PSUM") as ps_slab,
            tc.tile_pool(name="ps_av", bufs=2, space="PSUM") as ps_av,
            tc.tile_pool(name="ps_mm", bufs=1, space="PSUM") as ps_mm,
        ):
            identbf = consts.tile([128, 128], BF16)
            make_identity(nc, identbf)
            ones_bf = consts.tile([1, 128], BF16)
            nc.vector.memset(ones_bf, 1.0)

            wproj_sb = []
            for ci in range(CCH):
                t32 = xload.tile([128, 512], F32, tag="wload", name="wload")
                nc.gpsimd.dma_start(out=t32, in_=wproj[ci * 128 : (ci + 1) * 128, :])
                tb = consts.tile([128, 512], BF16, tag=f"wproj{ci}", name=f"wproj{ci}")
                nc.scalar.copy(tb, t32)
                wproj_sb.append(tb)
            bp32 = consts.tile([1, 512], F32)
            nc.gpsimd.dma_start(out=bp32, in_=bproj[:, :])
            bp_bf = consts.tile([1, 512], BF16)
            nc.vector.tensor_copy(bp_bf, bp32)

            for pr in range(2):
                # ---- kick off all x DMAs, then weight DMAs
                xt32s = []
                for nt in range(NT):
                    xt32 = xload.tile([128, 512], F32, tag="xload")
                    nc.gpsimd.dma_start(out=xt32, in_=x[pr, nt * 128 : (nt + 1) * 128, :])
                    xt32s.append(xt32)
                wq_sb, wk_sb, wv_sb = [], [], []
                w32s = []
                for ci in range(CCH):
                    rows = slice(ci * 128, (ci + 1) * 128)
                    for lst, wsrc in (
                        (wq_sb, wq[pr, rows, :]),
                        (wk_sb, wkv[pr, rows, 0:512]),
                        (wv_sb, wkv[pr, rows, 512:1024]),
                    ):
                        t32 = xload.tile([128, 512], F32, tag="wload", name="wload")
                        nc.gpsimd.dma_start(out=t32, in_=wsrc)
                        w32s.append(t32)

                # ---- xT [c-chunk, ci, n] via bf16 PE transpose
                xT = bigp.tile([128, CCH, N], BF16, tag="xTall", name="xTall")
                for nt in range(NT):
                    xtb = xload.tile([128, 512], BF16, tag="xbf")
                    nc.vector.tensor_copy(xtb, xt32s[nt])
                    for ci in range(CCH):
                        pst = ps_mm.tile([128, 128], BF16, tag=f"mm{pr}")
                        nc.tensor.transpose(
                            pst,
                            xtb[:, ci * 128 : (ci + 1) * 128],
                            identbf,
                        )
                        nc.vector.tensor_copy(
                            xT[:, ci, nt * 128 : (nt + 1) * 128], pst
                        )

                # ---- weight casts: wq/wk on DVE (gate qkT), wv on ACT
                for ci in range(CCH):
                    for j, (lst, tag, eng) in enumerate((
                        (wq_sb, f"wq{ci}", nc.vector),
                        (wk_sb, f"wk{ci}", nc.vector),
                        (wv_sb, f"wv{ci}", nc.scalar),
                    )):
                        t32 = w32s[3 * ci + j]
                        tb = wpool.tile([128, 512], BF16, tag=tag, name=tag)
                        if eng is nc.scalar:
                            nc.scalar.copy(tb, t32)
                        else:
                            eng.tensor_copy(tb, t32)
                        lst.append(tb)

                # ---- qT/kT [d, n] (d head-major: d-chunk di = heads 2di, 2di+1)
                qT = [bigp.tile([128, N], BF16, tag=f"qT{di}", name=f"qT{di}") for di in range(CCH)]
                kT = [bigp.tile([128, N], BF16, tag=f"kT{di}", name=f"kT{di}") for di in range(CCH)]
                for di in range(CCH):
                    dcols = slice(di * 128, (di + 1) * 128)
                    for nf in range(2):
                        ncols = slice(nf * 512, (nf + 1) * 512)
                        for dst, wsb in ((qT, wq_sb), (kT, wk_sb)):
                            ps = ps_mm.tile([128, 512], F32, tag=f"mm{pr}")
                            for ci in range(CCH):
                                nc.tensor.matmul(
                                    ps,
                                    wsb[ci][:, dcols],
                                    xT[:, ci, ncols],
                                    start=(ci == 0),
                                    stop=(ci == CCH - 1),
                                )
                            nc.vector.tensor_copy(dst[di][:, ncols], ps)

                # ---- v [m, d] padded with a ones column per head block
                vpad = [bigp.tile([128, HEADS * 65], BF16, tag=f"v{mt}", name=f"v{mt}") for mt in range(NT)]
                for mt in range(NT):
                    ps = ps_mm.tile([128, 512], F32, tag=f"mm{pr}")
                    for ci in range(CCH):
                        nc.tensor.matmul(
                            ps,
                            xT[:, ci, mt * 128 : (mt + 1) * 128],
                            wv_sb[ci],
                            start=(ci == 0),
                            stop=(ci == CCH - 1),
                        )
                    vv = vpad[mt].rearrange("p (h w) -> p h w", w=65)
                    nc.vector.memset(vv[:, :, 64:65], 1.0)
                    nc.vector.tensor_copy(
                        vv[:, :, 0:64], ps.rearrange("p (h w) -> p h w", w=64)
                    )

                # ---- attention: scores+exp per (di, mt); av streams et wide
                oT_sb = [otp.tile([128, N], BF16, tag=f"oT{di}", name=f"oT{di}") for di in range(CCH)]
                for di in range(CCH):
                    exps = [[None, None] for _ in range(NT)]
                    for mt in range(NT):
                        for half in range(2):
                            prow = slice(half * 64, (half + 1) * 64)
                            slab = ps_slab.tile([128, 1024], F32, tag="slab")
                            for nf in range(2):
                                nc.tensor.matmul(
                                    slab[:, nf * 512 : (nf + 1) * 512],
                                    kT[di][prow, mt * 128 : (mt + 1) * 128],
                                    qT[di][prow, nf * 512 : (nf + 1) * 512],
                                    start=True,
                                    stop=True,
                                )
                            et = expp.tile([128, 1024], BF16, tag="exp")
                            nc.scalar.activation(
                                et, slab, mybir.ActivationFunctionType.Exp, scale=0.125
                            )
                            exps[mt][half] = et

                    # av: out[d|den, n] = vpad_h.T @ et ; accumulate over mt.
                    # nf outer so the last di's nf-group unblocks proj of the
                    # matching n-tiles early. Denominator reciprocals are
                    # batched per (di, nf): the two halves' denom rows are
                    # PE-transposed into [128, 4] columns, reciprocal'd
                    # partition-parallel on DVE, and transposed back.
                    for nf in range(2):
                        avs = []
                        for half in range(2):
                            av = ps_av.tile([65, 512], F32, tag="av", name="av")
                            avs.append(av)
                        for mt in range(NT):
                            for half in range(2):
                                h = 2 * di + half
                                nc.tensor.matmul(
                                    avs[half],
                                    vpad[mt][:, h * 65 : (h + 1) * 65],
                                    exps[mt][half][
                                        :, nf * 512 : (nf + 1) * 512
                                    ],
                                    start=(mt == 0),
                                    stop=(mt == NT - 1),
                                )
                        dens = []
                        for half in range(2):
                            dn = smallp.tile([1, 512], BF16, tag=f"den{half}")
                            nc.vector.tensor_copy(dn, avs[half][64:65, :])
                            dens.append(dn)
                        rcT = ps_mm.tile([128, 16], BF16, tag=f"mm{pr}")
                        for half in range(2):
                            for k in range(4):
                                nc.tensor.transpose(
                                    rcT[:, 8 * half + 2 * k : 8 * half + 2 * k + 1],
                                    dens[half][:, 128 * k : 128 * (k + 1)],
                                    identbf[0:1, 0:1],
                                )
                        rcT_sb = smallp.tile([128, 8], BF16, tag="rcTsb")
                        with nc.allow_low_precision(
                            reason="bf16 softmax denom reciprocal"
                        ):
                            nc.vector.reciprocal(
                                rcT_sb,
                                rcT.rearrange("p (k two) -> p k two", two=2)[
                                    :, :, 0
                                ],
                            )
                        rc2s = []
                        for half in range(2):
                            r2 = ps_mm.tile([1, 512], BF16, tag=f"mm{pr}")
                            for k in range(4):
                                nc.tensor.transpose(
                                    r2[:, 128 * k : 128 * (k + 1)],
                                    rcT_sb[:, 4 * half + k : 4 * half + k + 1],
                                    identbf,
                                )
                            rc2s.append(r2)
                        for half in range(2):
                            rc = smallp.tile([1, 512], BF16, tag="rc")
                            nc.vector.tensor_copy(rc, rc2s[half])
                            repl = smallp.tile([64, 512], BF16, tag="repl")
                            nc.gpsimd.partition_broadcast(repl, rc)
                            nc.vector.scalar_tensor_tensor(
                                out=oT_sb[di][
                                    half * 64 : (half + 1) * 64,
                                    nf * 512 : (nf + 1) * 512,
                                ],
                                in0=avs[half][0:64, :],
                                scalar=1.0,
                                in1=repl,
                                op0=MUL,
                                op1=MUL,
                            )
                        # proj + bias for this nf's n-tiles once the last
                        # di's chains have landed
                        if di == CCH - 1:
                            for nt in range(nf * 4, nf * 4 + 4):
                                if pr == 0:
                                    zps = ps_mm.tile(
                                        [128, 512], F32, tag=f"mm{pr}",
                                        name="zps",
                                    )
                                else:
                                    zps = ps_slab.tile(
                                        [128, 512], F32, tag="slab",
                                        name="zps",
                                    )
                                nc.tensor.matmul(
                                    zps,
                                    ones_bf[0:1, :],
                                    bp_bf[0:1, :],
                                    start=True,
                                    stop=False,
                                )
                                for dj in range(CCH):
                                    nc.tensor.matmul(
                                        zps,
                                        oT_sb[dj][:, nt * 128 : (nt + 1) * 128],
                                        wproj_sb[dj],
                                        start=False,
                                        stop=(dj == CCH - 1),
                                    )
                                zsb = smallp.tile([128, 512], F32, tag="z")
                                nc.vector.tensor_copy(zsb, zps)
                                nc.gpsimd.dma_start(
                                    out=y[pr, nt * 128 : (nt + 1) * 128, :],
                                    in_=zsb,
                                )
    return nc


def _get_nc():
    if "nc" not in _CACHE:
        nc = _build_kernel()
        nc.compile()
        _CACHE["nc"] = nc
    return _CACHE["nc"]


def kernel(**inputs) -> np.ndarray:
    from concourse.bass_utils import run_bass_kernel_spmd

    x = np.ascontiguousarray(np.asarray(inputs["x"], dtype=np.float32))
    Wq = np.ascontiguousarray(np.asarray(inputs["Wq"], dtype=np.float32))
    Wkv = np.ascontiguousarray(np.asarray(inputs["Wkv"], dtype=np.float32))
    Wproj = np.ascontiguousarray(np.asarray(inputs["Wproj"], dtype=np.float32))
    bproj = np.ascontiguousarray(
        np.asarray(inputs["bproj"], dtype=np.float32).reshape(1, C)
    )

    nc = _get_nc()
    xr = x.reshape(B * P, N, C)
    in_maps = []
    for core in range(8):
        p0 = (2 * core) % P
        in_maps.append(
            {
                "x": np.ascontiguousarray(xr[2 * core : 2 * core + 2]),
                "wq": np.ascontiguousarray(Wq[p0 : p0 + 2]),
                "wkv": np.ascontiguousarray(Wkv[p0 : p0 + 2]),
                "wproj": Wproj,
                "bproj": bproj,
            }
        )
    res = run_bass_kernel_spmd(nc, in_maps, list(range(8))).results
    out = np.concatenate([r["y"] for r in res], axis=0).reshape(B, P, N, C)
    return out.astype(np.float32)
